# revision 20
# baseline (speedup 1.0000x reference)
"""CMC-V2 loss kernel for 8 Trainium2 NeuronCores (Bass/Tile).

Math
----
The reference loss decomposes into:
  - 9 NT-Xent contrastive terms. For pair (A, B) with row-normalized
    embeddings Z = [An; Bn] (N=4096 rows, D=512), the per-row sim matrix is
    sim = (Zn @ Zn.T)/0.2 = 5*cos.  Since rows are unit-norm, sim[i,i] = 5.0
    is the exact row max, so
        lse_i (diag excluded) = 5 + log(S_i - 1),  S_i = sum_j exp(5*cos_ij - 5)
    and sum_i pos_i = 10 * sum_i cos(An_i, Bn_i).
    per-pair loss = 5 + (1/4096) sum_i log(S_i - 1) - (10/4096) sum_i cos_i
  - 12 cosine-embedding terms: 1 - (1/2048) sum_i cos_i.
  Total constant: 9*5 + 12 = 57.

Sharding
--------
Data-parallel over 8 cores with a static SPMD program: core c receives every
input rolled by -256*c rows, so its shard is always rows [0:256) of each
matrix (matmul weights cannot take dynamic offsets). Each core:
  - normalizes all 12 half-matrices (bf16) and DMA-xbar-transposes them into
    ZnT layout [128 part = d%128, 4 = d//128, 2048 = sample],
  - computes its 512 Gram rows per pair (lhsT = its 256-row shard of A and B)
    against all 4096 columns; ScalarE applies exp(5x-5) with a fused
    per-row accumulate; log(S-1) summed on-chip,
  - computes its 256-row shard of the 21 row-dot (cosine) sums,
  - returns partial sums in a [128, 4] f32 tensor.
Host sums the 8 partials and applies the closed-form combination.
"""

import numpy as np
import ml_dtypes
from contextlib import ExitStack

from concourse import bass, bacc, tile, mybir
from concourse.bass_utils import run_bass_kernel_spmd

BF16 = mybir.dt.bfloat16
FP8 = mybir.dt.float8e4
F32 = mybir.dt.float32
AF = mybir.ActivationFunctionType
ALU = mybir.AluOpType

# fp8 variant: Gram matmuls in fp8e4m3 with DoubleRow (2 MACs/cell/cycle).
# Normalized rows are pre-scaled by 16 so fp8 sees values ~N(0, 0.71^2);
# the Gram then yields 256*cos and the exp scale becomes 5/256.
USE_FP8 = False
FP8_SCALE = 16.0

B = 2048          # batch
DH = 512          # half feature dim
N_CORES = 8
R = B // N_CORES  # 256 rows per core shard
NT = B // 128     # 16 row tiles per half-matrix
KC = DH // 128    # 4 contraction chunks
CBW = 512         # column block width
CB = B // CBW     # 4 col blocks per matrix

NAMES = ["f1_m0", "f1_m1", "f1_m2", "f2_m0", "f2_m1", "f2_m2"]

# contrastive pairs as ((f, h), (f, h)); h: 0 = shared, 1 = private
PAIRS_S1 = [((0, 0), (1, 0)), ((0, 0), (2, 0)), ((1, 0), (2, 0))]
PAIRS_S2 = [((3, 0), (4, 0)), ((3, 0), (5, 0)), ((4, 0), (5, 0))]
PAIRS_P = [((0, 1), (3, 1)), ((1, 1), (4, 1)), ((2, 1), (5, 1))]
ORTHO_V1 = [((0, 0), (0, 1)), ((1, 0), (1, 1)), ((2, 0), (2, 1)),
            ((0, 1), (1, 1)), ((0, 1), (2, 1)), ((1, 1), (2, 1))]
ORTHO_V2 = [((3, 0), (3, 1)), ((4, 0), (4, 1)), ((5, 0), (5, 1)),
            ((3, 1), (4, 1)), ((3, 1), (5, 1)), ((4, 1), (5, 1))]

N_SLOTS = 9 * 4   # 9 pairs x 4 M-tiles of 128 Gram rows each
N_DOTS = 21       # 9 contrastive + 12 ortho row-dot sums


def build_program(use_fp8=USE_FP8, only_build=False, act_accum=True,
                  dma_transpose=True):
    nc = bacc.Bacc(
        "TRN2",
        target_bir_lowering=False,
        debug=False,
        enable_asserts=False,
        num_devices=N_CORES,
    )
    ffs = [nc.dram_tensor(n, [B, 2 * DH], BF16, kind="ExternalInput").ap()
           for n in NAMES]
    out_dram = nc.dram_tensor("part", [128, 4], F32, kind="ExternalOutput").ap()

    with tile.TileContext(nc) as tc, ExitStack() as ctx:
        znt_pool = ctx.enter_context(tc.tile_pool(name="zntp", bufs=6))
        x_pool = ctx.enter_context(tc.tile_pool(name="xp", bufs=6))
        zn_pool = ctx.enter_context(tc.tile_pool(name="znp", bufs=4))
        vscr_pool = ctx.enter_context(tc.tile_pool(name="vscrp", bufs=3))
        escr_pool = ctx.enter_context(tc.tile_pool(name="escrp", bufs=3))
        nrm_pool = ctx.enter_context(tc.tile_pool(name="nrmp", bufs=2))
        sab_pool = ctx.enter_context(tc.tile_pool(name="sabp", bufs=4))
        acc_pool = ctx.enter_context(tc.tile_pool(name="accp", bufs=1))
        psum_pool = ctx.enter_context(
            tc.tile_pool(name="psump", bufs=2, space="PSUM"))

        biasm5 = acc_pool.tile([128, 1], F32, tag="biasm5", name="biasm5")
        nc.gpsimd.memset(biasm5[:], -5.0)
        if not dma_transpose:
            ident = acc_pool.tile([128, 128], BF16, tag="ident", name="ident")
            iota_r = acc_pool.tile([128, 128], F32, tag="iota_r", name="iota_r")
            iota_p = acc_pool.tile([128, 1], F32, tag="iota_p", name="iota_p")
            nc.gpsimd.iota(iota_r[:], pattern=[[1, 128]], base=0,
                           channel_multiplier=0,
                           allow_small_or_imprecise_dtypes=True)
            nc.gpsimd.iota(iota_p[:], pattern=[[0, 1]], base=0,
                           channel_multiplier=1,
                           allow_small_or_imprecise_dtypes=True)
            nc.vector.tensor_scalar(
                out=ident[:], in0=iota_r[:], scalar1=iota_p[:, 0:1],
                scalar2=None, op0=ALU.is_equal)
        sm1 = acc_pool.tile([128, N_SLOTS], F32, tag="sm1", name="sm1")
        dots_all = acc_pool.tile([128, N_DOTS], F32, tag="dots", name="dots_all")
        logv = acc_pool.tile([128, N_SLOTS], F32, tag="logv", name="logv")
        part = acc_pool.tile([128, 4], F32, tag="part", name="part_sb")

        znt = {}

        def build_half(f, h):
            """Normalize rows of half-matrix (f, h) and store transposed
            ZnT tile [128, KC, B]: znt[p, c, j] = Zn[j, c*128 + p].
            bf16 variant stores Zn directly; fp8 stores 16*Zn as fp8e4m3
            (via a transient bf16 transposed tile, since the DMA xbar
            transpose requires a 2-byte dtype)."""
            if use_fp8:
                zt = znt_pool.tile([128, KC, B], BF16, tag="zntb", bufs=2,
                                   name=f"zntb{f}_{h}")
            else:
                zt = znt_pool.tile([128, KC, B], BF16, tag="znt", name=f"znt{f}_{h}")
            norms = nrm_pool.tile([128, NT], F32, tag="norms", name=f"nrm{f}_{h}")
            sqn = nrm_pool.tile([128, NT], F32, tag="sqn", name=f"sqn{f}_{h}")
            rinv = nrm_pool.tile([128, NT], F32, tag="rinv", name=f"rinv{f}_{h}")
            for g in range(NT // 4):
                xts = []
                for t in range(4 * g, 4 * g + 4):
                    xt = x_pool.tile([128, DH], BF16, tag="xt", name=f"xt{f}_{h}_{t}")
                    nc.gpsimd.dma_start(
                        out=xt[:],
                        in_=ffs[f][t * 128:(t + 1) * 128, h * DH:(h + 1) * DH])
                    xts.append(xt)
                    sq = vscr_pool.tile([128, DH], F32, tag="vscr",
                                        name=f"sq{f}_{h}_{t}")
                    # sumsq: one DVE pass, accum_out = sum((x*1)*x)
                    nc.vector.scalar_tensor_tensor(
                        out=sq[:], in0=xt[:], scalar=1.0, in1=xt[:],
                        op0=ALU.mult, op1=ALU.mult,
                        accum_out=norms[:, t:t + 1])
                cs = slice(4 * g, 4 * g + 4)
                nc.scalar.activation(sqn[:, cs], norms[:, cs], AF.Sqrt)
                nc.vector.reciprocal(rinv[:, cs], sqn[:, cs])
                for i, t in enumerate(range(4 * g, 4 * g + 4)):
                    zn = zn_pool.tile([128, DH], BF16, tag="zn",
                                      name=f"zn{f}_{h}_{t}")
                    if use_fp8:
                        nc.vector.tensor_scalar(
                            out=zn[:], in0=xts[i][:],
                            scalar1=rinv[:, t:t + 1], scalar2=FP8_SCALE,
                            op0=ALU.mult, op1=ALU.mult)
                    else:
                        nc.vector.tensor_scalar_mul(
                            out=zn[:], in0=xts[i][:], scalar1=rinv[:, t:t + 1])
                    if dma_transpose:
                        nc.sync.dma_start(
                            out=zt[:, :, t * 128:(t + 1) * 128], in_=zn[:],
                            transpose=True)
                    else:
                        for c in range(KC):
                            tp = psum_pool.tile([128, 128], BF16, tag="tpp",
                                                bufs=2, name=f"tp{f}_{h}_{t}_{c}")
                            nc.tensor.transpose(
                                tp[:], zn[:, c * 128:(c + 1) * 128], ident[:])
                            nc.scalar.copy(
                                zt[:, c, t * 128:(t + 1) * 128], tp[:])
            if use_fp8:
                z8 = znt_pool.tile([128, KC, B], FP8, tag="znt", name=f"znt8{f}_{h}")
                nc.vector.tensor_copy(z8[:], zt[:])
                znt[(f, h)] = z8
            else:
                znt[(f, h)] = zt

        slot_i = 0

        def gram(A, Bm):
            """Emit Gram+exp+rowsum for contrastive pair (A, Bm)."""
            nonlocal slot_i
            for X in (A, Bm):          # lhsT source (core's 256-row shard)
                for mt in range(2):    # two 128-row M tiles
                    sab = sab_pool.tile([128, 2], F32, tag="sab",
                                        name=f"sab{slot_i}")
                    for ridx, RH in enumerate((A, Bm)):   # rhs matrix
                        ps = psum_pool.tile([128, CB, CBW], F32, tag="gram",
                                            name=f"ps{slot_i}_{ridx}")
                        for cb in range(CB):
                            if use_fp8:
                                for q in range(KC // 2):
                                    nc.tensor.matmul(
                                        ps[:, cb, :],
                                        znt[X][:, 2 * q:2 * q + 2,
                                               mt * 128:(mt + 1) * 128],
                                        znt[RH][:, 2 * q:2 * q + 2,
                                                cb * CBW:(cb + 1) * CBW],
                                        perf_mode=mybir.MatmulPerfMode.DoubleRow,
                                        start=(q == 0), stop=(q == KC // 2 - 1))
                            else:
                                for kc in range(KC):
                                    nc.tensor.matmul(
                                        ps[:, cb, :],
                                        znt[X][:, kc, mt * 128:(mt + 1) * 128],
                                        znt[RH][:, kc, cb * CBW:(cb + 1) * CBW],
                                        start=(kc == 0), stop=(kc == KC - 1))
                        es = escr_pool.tile([128, CB, CBW], BF16, tag="escr",
                                            name=f"es{slot_i}_{ridx}")
                        exp_scale = 5.0 / (FP8_SCALE * FP8_SCALE) if use_fp8 else 5.0
                        if act_accum:
                            nc.scalar.activation(
                                es[:], ps[:], AF.Exp, bias=biasm5[:],
                                scale=exp_scale,
                                accum_out=sab[:, ridx:ridx + 1])
                        else:
                            nc.scalar.activation(
                                es[:], ps[:], AF.Exp, bias=biasm5[:],
                                scale=exp_scale)
                            nc.vector.tensor_reduce(
                                sab[:, ridx:ridx + 1], es[:],
                                axis=mybir.AxisListType.XY, op=ALU.add)
                    # sm1[:, slot] = (S_A - 0.5) + (S_B - 0.5) = S - 1
                    scr2 = sab_pool.tile([128, 2], F32, tag="scr2",
                                         name=f"scr2_{slot_i}")
                    if act_accum:
                        nc.vector.tensor_scalar(
                            out=scr2[:], in0=sab[:], scalar1=-0.5, scalar2=None,
                            op0=ALU.add, op1=ALU.add,
                            accum_out=sm1[:, slot_i:slot_i + 1])
                    else:
                        nc.vector.tensor_scalar(
                            out=scr2[:], in0=sab[:], scalar1=-0.5, scalar2=None,
                            op0=ALU.add)
                        nc.vector.tensor_reduce(
                            sm1[:, slot_i:slot_i + 1], scr2[:],
                            axis=mybir.AxisListType.X, op=ALU.add)
                    slot_i += 1

        def dots(col, X, Y):
            """dots_all[:, col] = per-partition sum over the core's 256-row
            shard of <Zn_X[i], Zn_Y[i]> (row-wise cosines)."""
            o = vscr_pool.tile([128, KC, R], F32, tag="vscr", name=f"do{col}")
            dscale = 1.0 / (FP8_SCALE * FP8_SCALE) if use_fp8 else 1.0
            nc.vector.scalar_tensor_tensor(
                out=o[:], in0=znt[X][:, :, 0:R], scalar=dscale,
                in1=znt[Y][:, :, 0:R], op0=ALU.mult, op1=ALU.mult,
                accum_out=dots_all[:, col:col + 1])

        # dots_all columns: 0..8 contrastive (s1 x3, p x3, s2 x3), 9..20 ortho.
        # Emission is grouped so every znt tile's last access stays within its
        # residency group (tile slots are recycled across groups).

        # ---- Group 1: view-1 matrices (6 tiles: s1_*, p1_*) ----
        for f in (0, 1, 2):
            build_half(f, 0)
            build_half(f, 1)
        if not only_build:
            for A, Bm in PAIRS_S1:
                gram(A, Bm)
        for i, (A, Bm) in enumerate(PAIRS_S1):
            dots(i, A, Bm)
        for i, (A, Bm) in enumerate(ORTHO_V1):
            dots(9 + i, A, Bm)
        # s1_* tiles free here; p1_* stay.

        # ---- Group 2a: private view-2 (p2_* into s1 slots) ----
        for f in (3, 4, 5):
            build_half(f, 1)
        if not only_build:
            for A, Bm in PAIRS_P:
                gram(A, Bm)
        for i, (A, Bm) in enumerate(PAIRS_P):
            dots(3 + i, A, Bm)
        # p1_* tiles free here; p2_* stay.

        # ---- Group 2b: shared view-2 (s2_* into p1 slots) ----
        for f in (3, 4, 5):
            build_half(f, 0)
        if not only_build:
            for A, Bm in PAIRS_S2:
                gram(A, Bm)
        for i, (A, Bm) in enumerate(PAIRS_S2):
            dots(6 + i, A, Bm)
        for i, (A, Bm) in enumerate(ORTHO_V2):
            dots(15 + i, A, Bm)

        # ---- epilogue ----
        if only_build:
            nc.gpsimd.memset(sm1[:], 1.0)
        nc.scalar.activation(logv[:], sm1[:], AF.Ln)
        nc.vector.memset(part[:], 0.0)
        nc.vector.tensor_reduce(part[:, 0:1], logv[:], axis=mybir.AxisListType.X,
                                op=ALU.add)
        nc.vector.tensor_reduce(part[:, 1:2], dots_all[:, 0:9],
                                axis=mybir.AxisListType.X, op=ALU.add)
        nc.vector.tensor_reduce(part[:, 2:3], dots_all[:, 9:21],
                                axis=mybir.AxisListType.X, op=ALU.add)
        nc.sync.dma_start(out=out_dram, in_=part[:])

    nc.compile()
    return nc


_PROG = None


def _get_prog():
    global _PROG
    if _PROG is None:
        _PROG = build_program()
    return _PROG


def make_in_maps(inputs):
    bf = ml_dtypes.bfloat16
    in_maps = []
    for c in range(N_CORES):
        m = {}
        for n in NAMES:
            a = np.asarray(inputs[n], dtype=np.float32)
            m[n] = np.ascontiguousarray(np.roll(a, -R * c, axis=0)).astype(bf)
        in_maps.append(m)
    return in_maps


def combine(parts):
    """parts: list of 8 [128, 4] f32 arrays -> scalar loss."""
    tl = tcc = toc = 0.0
    for p in parts:
        p = np.asarray(p, dtype=np.float64)
        tl += p[:, 0].sum()
        tcc += p[:, 1].sum()
        toc += p[:, 2].sum()
    n2 = float(2 * B)
    loss = (9 * 5.0 + 12.0) + tl / n2 - 10.0 * tcc / n2 - toc / float(B)
    return np.float32(loss)


def kernel(**inputs):
    nc = _get_prog()
    in_maps = make_in_maps(inputs)
    res = run_bass_kernel_spmd(nc, in_maps, list(range(N_CORES)))
    return combine([res.results[c]["part"] for c in range(N_CORES)])
